# revision 12
# baseline (speedup 1.0000x reference)
"""MoE layer (dense all-expert routing) Trainium2 Bass kernel.

Problem: x[4,2048,1024] f32, gate_w[1024,8], gate_b[8], expert_w[8,1024,1024].
  gate = softmax(x @ gate_w + gate_b)                  # [B,S,E]
  out  = einsum('bse,bseo->bso', gate, einsum('bsi,eio->bseo', x, expert_w))

Sharding: data-parallel over tokens. 8192 tokens split into 8 shards of 1024;
each core computes its shard against all 8 experts (weights replicated).
No collectives; host concatenates shard outputs.

Per-core kernel (strategy: fused gate-scaled PSUM combine):
  - xT (d_in-major) and all expert weights streamed as bf16; PSUM accumulates f32.
  - Gate logits computed on PE in [token, expert] orientation so softmax is
    all free-dim ops; bias added via a partition-broadcast DMA of gate_b.
  - Main loop: for each (n-half, expert, k-tile) stream W tile, matmul into
    per-m PSUM tiles (accumulate over k); per expert, fold into SBUF
    accumulator with one fused DVE op: acc = (psum * g[:,e]) + acc.
"""

import numpy as np
import ml_dtypes
from contextlib import ExitStack

import concourse.bacc as bacc
import concourse.bass as bass
import concourse.mybir as mybir
import concourse.tile as tile

BF16 = mybir.dt.bfloat16
F32 = mybir.dt.float32

P = 128  # partitions


def build_moe_nc(T=1024, D=1024, O=1024, E=8, NO=512, w_bufs=3, acc_bufs=16):
    """Build the per-core Bass program.

    T: tokens per core, D: d_in, O: d_out, E: experts, NO: d_out tile (<=512).
    """
    KT = D // P   # k tiles (contraction)
    MT = T // P   # token tiles
    NT = O // NO  # d_out tiles

    nc = bacc.Bacc("TRN2", target_bir_lowering=False, debug=False)
    xT_d = nc.dram_tensor("xT", [D, T], BF16, kind="ExternalInput")
    w_d = nc.dram_tensor("w", [E, D, O], BF16, kind="ExternalInput")
    gw_d = nc.dram_tensor("gw", [D, E], BF16, kind="ExternalInput")
    gb_d = nc.dram_tensor("gb", [1, E], F32, kind="ExternalInput")
    out_d = nc.dram_tensor("out", [T, O], F32, kind="ExternalOutput")

    with tile.TileContext(nc) as tc:
        with ExitStack() as ctx:
            singles = ctx.enter_context(tc.tile_pool(name="singles", bufs=1))
            wpool = ctx.enter_context(tc.tile_pool(name="w", bufs=w_bufs))
            accp = ctx.enter_context(tc.tile_pool(name="acc", bufs=acc_bufs))
            gpool = ctx.enter_context(tc.tile_pool(name="gate", bufs=1))
            ps = ctx.enter_context(
                tc.tile_pool(name="ps", bufs=8, space="PSUM")
            )

            # ---- resident loads -------------------------------------------
            # Input loads go on the ACT HWDGE ring (nc.scalar) so the bulk
            # weight stream on the sync ring is never blocked behind them.
            gb_b = singles.tile([P, E], F32, tag="gb")
            nc.scalar.dma_start(out=gb_b, in_=gb_d[0:1, :].to_broadcast([P, E]))
            gw_t = singles.tile([P, KT, E], BF16, tag="gw")
            nc.scalar.dma_start(
                out=gw_t, in_=gw_d[:, :].rearrange("(k p) e -> p k e", p=P)
            )
            gw = [gw_t[:, k, :] for k in range(KT)]
            # per-k xT DMAs: lets gate matmuls start as soon as xT[0] lands
            xT = []
            for k in range(KT):
                t = singles.tile([P, T], BF16, tag=f"xT{k}")
                nc.scalar.dma_start(out=t, in_=xT_d[k * P:(k + 1) * P, :])
                xT.append(t)

            # ---- gate: logits -> softmax, [token, expert] orientation -----
            g_sb = []  # per m-tile: [P, E] f32 normalized gate weights
            for m in range(MT):
                psg = ps.tile([P, NO], F32, tag="ps")
                for k in range(KT):
                    nc.tensor.matmul(
                        psg[:, 0:E],
                        lhsT=xT[k][:, m * P:(m + 1) * P],
                        rhs=gw[k],
                        start=(k == 0),
                        stop=(k == KT - 1),
                    )
                lg = gpool.tile([P, E], F32, tag=f"lg{m}")
                nc.vector.tensor_add(lg, psg[:, 0:E], gb_b)
                p_t = gpool.tile([P, E], F32, tag=f"p{m}")
                s_t = gpool.tile([P, 1], F32, tag=f"s{m}")
                # exp(logits); |logits| <~ 3 so no max-subtraction needed
                nc.scalar.activation(
                    p_t, lg, mybir.ActivationFunctionType.Exp, accum_out=s_t
                )
                rs_t = gpool.tile([P, 1], F32, tag=f"rs{m}")
                nc.vector.reciprocal(rs_t, s_t)
                g_t = gpool.tile([P, E], F32, tag=f"g{m}")
                nc.vector.tensor_scalar_mul(g_t, p_t, rs_t)
                g_sb.append(g_t)

            # ---- main: all-expert GEMM + fused gate combine ---------------
            for n in range(NT):
                acc = [None] * MT
                for e in range(E):
                    # one 1MiB DMA per (n, e): all k-tiles of this d_out slice
                    wt = wpool.tile([P, KT, NO], BF16, tag="w")
                    nc.sync.dma_start(
                        out=wt,
                        in_=w_d[e, :, n * NO:(n + 1) * NO].rearrange(
                            "(k p) o -> p k o", p=P
                        ),
                    )
                    psy = [None] * MT
                    for k in range(KT):
                        for m in range(MT):
                            if k == 0:
                                psy[m] = ps.tile(
                                    [P, NO], F32, tag="ps", name=f"psy{m}"
                                )
                            nc.tensor.matmul(
                                psy[m],
                                lhsT=xT[k][:, m * P:(m + 1) * P],
                                rhs=wt[:, k, :],
                                start=(k == 0),
                                stop=(k == KT - 1),
                            )
                    for m in range(MT):
                        if e == 0:
                            acc[m] = accp.tile(
                                [P, NO], F32, tag="acc", name=f"acc{m}"
                            )
                            nc.vector.tensor_scalar_mul(
                                acc[m], psy[m], g_sb[m][:, 0:1]
                            )
                        else:
                            nc.vector.scalar_tensor_tensor(
                                out=acc[m],
                                in0=psy[m],
                                scalar=g_sb[m][:, e:e + 1],
                                in1=acc[m],
                                op0=mybir.AluOpType.mult,
                                op1=mybir.AluOpType.add,
                            )
                for m in range(MT):
                    nc.scalar.dma_start(
                        out=out_d[m * P:(m + 1) * P, n * NO:(n + 1) * NO],
                        in_=acc[m],
                    )
    nc.compile()
    return nc


# ---------------------------------------------------------------------------
# Host wrapper: full inputs -> shard -> run SPMD on 8 cores -> gather
# ---------------------------------------------------------------------------

N_CORES = 8
_B, _S, _DIN, _DOUT, _E = 4, 2048, 1024, 1024, 8


LAST_RESULTS = None  # BassKernelResults of the most recent run (for profiling)


def kernel(x, gate_w, gate_b, expert_w, _trace=False):
    global LAST_RESULTS
    from concourse.bass_utils import run_bass_kernel_spmd

    x = np.asarray(x)
    tokens = x.reshape(-1, _DIN)  # [8192, 1024]
    n_tok = tokens.shape[0]
    tpc = n_tok // N_CORES  # tokens per core

    w_bf = np.asarray(expert_w, dtype=ml_dtypes.bfloat16)
    gw_bf = np.asarray(gate_w, dtype=ml_dtypes.bfloat16)
    gb_f = np.asarray(gate_b, dtype=np.float32).reshape(1, _E)

    in_maps = []
    for c in range(N_CORES):
        shard = tokens[c * tpc:(c + 1) * tpc]  # [1024, 1024]
        xT = np.ascontiguousarray(shard.T).astype(ml_dtypes.bfloat16)
        in_maps.append({"xT": xT, "w": w_bf, "gw": gw_bf, "gb": gb_f})

    nc = build_moe_nc(T=tpc, D=_DIN, O=_DOUT, E=_E)
    res = run_bass_kernel_spmd(nc, in_maps, list(range(N_CORES)), trace=_trace)
    LAST_RESULTS = res
    outs = [res.results[c]["out"] for c in range(N_CORES)]
    full = np.concatenate(outs, axis=0).astype(np.float32)
    return full.reshape(_B, _S, _DOUT)


# revision 13
# speedup vs baseline: 1.0327x; 1.0327x over previous
"""MoE layer (dense all-expert routing) Trainium2 Bass kernel.

Problem: x[4,2048,1024] f32, gate_w[1024,8], gate_b[8], expert_w[8,1024,1024].
  gate = softmax(x @ gate_w + gate_b)                  # [B,S,E]
  out  = einsum('bse,bseo->bso', gate, einsum('bsi,eio->bseo', x, expert_w))

Sharding: data-parallel over tokens. 8192 tokens split into 8 shards of 1024;
each core computes its shard against all 8 experts (weights replicated).
No collectives; host concatenates shard outputs.

Per-core kernel:
  - all matmuls bf16 with f32 PSUM accumulation (rel err ~3e-3)
  - gate logits on PE in [token, expert] orientation; gate_b added via a
    K=1 ones-matmul into the same PSUM accumulation group; softmax is then
    all free-dim ops (exp w/ accum_out, reciprocal, scale)
  - gate uses its own 1-slot PSUM tag; main loop uses 7 slots so the first
    expert's matmuls start as soon as x/W tiles land (no gate dependency)
  - main loop: per (n-half, expert) one 1MiB weight DMA, 64 matmuls,
    then per m one fused DVE op: acc = (psum * g[:,e]) + acc
  - input DMAs split across both HWDGE rings (sync + scalar) for latency
"""

import numpy as np
import ml_dtypes
from contextlib import ExitStack

import concourse.bacc as bacc
import concourse.bass as bass
import concourse.mybir as mybir
import concourse.tile as tile

BF16 = mybir.dt.bfloat16
F32 = mybir.dt.float32

P = 128  # partitions


def build_moe_nc(T=1024, D=1024, O=1024, E=8, NO=512, w_bufs=3, acc_bufs=16):
    """Build the per-core Bass program.

    T: tokens per core, D: d_in, O: d_out, E: experts, NO: d_out tile (<=512).
    """
    KT = D // P   # k tiles (contraction)
    MT = T // P   # token tiles
    NT = O // NO  # d_out tiles
    KH = KT // 4  # k tiles per xT load chunk (4 chunks)

    nc = bacc.Bacc("TRN2", target_bir_lowering=False, debug=False)
    xT_d = nc.dram_tensor("xT", [D, T], BF16, kind="ExternalInput")
    w_d = nc.dram_tensor("w", [E, D, O], BF16, kind="ExternalInput")
    # gwt[p, k*E+e] = gate_w[k*128+p, e]  (host pre-tiled, contiguous DMA)
    gwt_d = nc.dram_tensor("gwt", [P, KT * E], BF16, kind="ExternalInput")
    gb_d = nc.dram_tensor("gb", [1, E], BF16, kind="ExternalInput")
    out_d = nc.dram_tensor("out", [T, O], F32, kind="ExternalOutput")

    with tile.TileContext(nc) as tc:
        with ExitStack() as ctx:
            singles = ctx.enter_context(tc.tile_pool(name="singles", bufs=1))
            wpool = ctx.enter_context(tc.tile_pool(name="w", bufs=w_bufs))
            accp = ctx.enter_context(tc.tile_pool(name="acc", bufs=acc_bufs))
            gpool = ctx.enter_context(tc.tile_pool(name="gate", bufs=1))
            ps = ctx.enter_context(tc.tile_pool(name="ps", bufs=7, space="PSUM"))

            # ---- resident loads -------------------------------------------
            # xT in 4 chunks: k-lo half on the sync ring (with the weights),
            # k-hi half on the ACT ring, so first matmuls start early.
            xc = [None] * 4
            for c in range(4):
                eng = nc.sync if c < 2 else nc.scalar
                xc[c] = singles.tile([P, KH, T], BF16, tag=f"xT{c}", name=f"xc{c}")
                eng.dma_start(
                    out=xc[c],
                    in_=xT_d[c * KH * P:(c + 1) * KH * P, :].rearrange(
                        "(k p) t -> p k t", p=P
                    ),
                )

            def xT(k):
                return xc[k // KH][:, k % KH, :]

            gb_sb = singles.tile([1, E], BF16, tag="gb")
            nc.scalar.dma_start(out=gb_sb, in_=gb_d[:, :])
            gw_t = singles.tile([P, KT, E], BF16, tag="gw")
            nc.scalar.dma_start(
                out=gw_t, in_=gwt_d[:, :].rearrange("p (k e) -> p k e", e=E)
            )
            ones_t = singles.tile([1, P], BF16, tag="ones")
            nc.vector.memset(ones_t, 1.0)

            # ---- main: all-expert GEMM + fused gate combine ---------------
            # (emitted before the gate so its instructions get priority; the
            # gate only gates the per-expert combine, not the matmuls)
            g_sb = [None] * MT  # filled by the gate section below

            def emit_gate():
                for m in range(MT):
                    psg = ps.tile([P, E], F32, tag="psg", bufs=1, name=f"psg{m}")
                    nc.tensor.matmul(
                        psg, lhsT=ones_t, rhs=gb_sb, start=True, stop=False
                    )
                    for k in range(KT):
                        nc.tensor.matmul(
                            psg,
                            lhsT=xT(k)[:, m * P:(m + 1) * P],
                            rhs=gw_t[:, k, :],
                            start=False,
                            stop=(k == KT - 1),
                        )
                    p_t = gpool.tile([P, E], F32, tag=f"p{m}", name=f"p{m}")
                    s_t = gpool.tile([P, 1], F32, tag=f"s{m}", name=f"s{m}")
                    # exp(logits); |logits| <~ 3 so no max-subtraction needed
                    nc.scalar.activation(
                        p_t, psg, mybir.ActivationFunctionType.Exp,
                        accum_out=s_t,
                    )
                    rs_t = gpool.tile([P, 1], F32, tag=f"rs{m}", name=f"rs{m}")
                    nc.vector.reciprocal(rs_t, s_t)
                    g_t = gpool.tile([P, E], F32, tag=f"g{m}", name=f"g{m}")
                    nc.vector.tensor_scalar_mul(g_t, p_t, rs_t)
                    g_sb[m] = g_t

            gate_emitted = False
            for n in range(NT):
                acc = [None] * MT
                for e in range(E):
                    # one 1MiB DMA per (n, e): all k-tiles of this d_out
                    # slice (the very first is split so k0-3 land earlier)
                    wt = wpool.tile([P, KT, NO], BF16, tag="w")
                    w_src = w_d[e, :, n * NO:(n + 1) * NO].rearrange(
                        "(k p) o -> p k o", p=P
                    )
                    if n == 0 and e == 0:
                        nc.sync.dma_start(
                            out=wt[:, 0:KT // 2, :], in_=w_src[:, 0:KT // 2, :]
                        )
                        nc.sync.dma_start(
                            out=wt[:, KT // 2:, :], in_=w_src[:, KT // 2:, :]
                        )
                    else:
                        nc.sync.dma_start(out=wt, in_=w_src)
                    psy = [None] * MT
                    for k in range(KT):
                        for m in range(MT):
                            if k == 0:
                                psy[m] = ps.tile(
                                    [P, NO], F32, tag="ps", name=f"psy{m}"
                                )
                            nc.tensor.matmul(
                                psy[m],
                                lhsT=xT(k)[:, m * P:(m + 1) * P],
                                rhs=wt[:, k, :],
                                start=(k == 0),
                                stop=(k == KT - 1),
                            )
                    if not gate_emitted:
                        # gate instructions come after the first expert's
                        # matmuls in program order but run concurrently
                        # (own PSUM slot); needed before the first combine
                        emit_gate()
                        gate_emitted = True
                    for m in range(MT):
                        if e == 0:
                            acc[m] = accp.tile(
                                [P, NO], F32, tag="acc", name=f"acc{m}"
                            )
                            nc.vector.tensor_scalar_mul(
                                acc[m], psy[m], g_sb[m][:, 0:1]
                            )
                        else:
                            nc.vector.scalar_tensor_tensor(
                                out=acc[m],
                                in0=psy[m],
                                scalar=g_sb[m][:, e:e + 1],
                                in1=acc[m],
                                op0=mybir.AluOpType.mult,
                                op1=mybir.AluOpType.add,
                            )
                for m in range(MT):
                    nc.scalar.dma_start(
                        out=out_d[m * P:(m + 1) * P, n * NO:(n + 1) * NO],
                        in_=acc[m],
                    )
    nc.compile()
    return nc


# ---------------------------------------------------------------------------
# Host wrapper: full inputs -> shard -> run SPMD on 8 cores -> gather
# ---------------------------------------------------------------------------

N_CORES = 8
_B, _S, _DIN, _DOUT, _E = 4, 2048, 1024, 1024, 8


def _host_gwt(gate_w):
    """[D, E] -> [128, KT*E] with gwt[p, k*E+e] = gate_w[k*128+p, e]."""
    D, E = gate_w.shape
    kt = D // P
    return np.ascontiguousarray(
        gate_w.reshape(kt, P, E).transpose(1, 0, 2).reshape(P, kt * E)
    )


LAST_RESULTS = None  # BassKernelResults of the most recent run (for profiling)


def kernel(x, gate_w, gate_b, expert_w, _trace=False):
    global LAST_RESULTS
    from concourse.bass_utils import run_bass_kernel_spmd

    x = np.asarray(x)
    tokens = x.reshape(-1, _DIN)  # [8192, 1024]
    n_tok = tokens.shape[0]
    tpc = n_tok // N_CORES  # tokens per core

    w_bf = np.asarray(expert_w, dtype=ml_dtypes.bfloat16)
    gwt_bf = _host_gwt(np.asarray(gate_w)).astype(ml_dtypes.bfloat16)
    gb_bf = np.asarray(gate_b, dtype=np.float32).reshape(1, _E).astype(
        ml_dtypes.bfloat16
    )

    in_maps = []
    for c in range(N_CORES):
        shard = tokens[c * tpc:(c + 1) * tpc]  # [1024, 1024]
        xT = np.ascontiguousarray(shard.T).astype(ml_dtypes.bfloat16)
        in_maps.append({"xT": xT, "w": w_bf, "gwt": gwt_bf, "gb": gb_bf})

    nc = build_moe_nc(T=tpc, D=_DIN, O=_DOUT, E=_E)
    res = run_bass_kernel_spmd(nc, in_maps, list(range(N_CORES)), trace=_trace)
    LAST_RESULTS = res
    outs = [res.results[c]["out"] for c in range(N_CORES)]
    full = np.concatenate(outs, axis=0).astype(np.float32)
    return full.reshape(_B, _S, _DOUT)
